# revision 48
# baseline (speedup 1.0000x reference)
"""GCNN message-passing kernel for Trainium2 (8 NeuronCores, batch-parallel).

Reference computation per graph:
    ax  = segment_sum(vals[:, None] * x[cols], rows, N)   # sparse A @ x
    out = relu(ax @ W + b)

Sharding: one graph per NeuronCore (data parallel over batch, W/b replicated).

Per-core device strategy (KCFG defaults: stream + dedup + balance):
  Host prep (index marshaling only):
    - balance_perm: snake-renumber destination nodes by in-degree so every
      128-node dst tile gets ~equal edge counts (evens out per-queue
      desc-gen work; output rows un-permuted on the host afterwards).
    - Per dst tile, collect the UNIQUE source nodes (~3.4k of 4.1k edges);
      each unique source occupies one gather slot. colsw holds the slot ->
      source-row indices (int16, dma_gather wrapped layout), padded with -1
      in each tile's last chunk (the Q7 trims trailing negatives, so pad
      slots cost no descriptors).
    - rw: fp16 multi-hot scatter matrices, 128 cols per chunk; row s of
      chunk k accumulates vals of all edges from that slot's source into
      dst offset m. Streamed from HBM at run time (HWDGE; building these
      on-chip with DVE tensor_scalar stalls GPSIMD desc-gen: the scalar-
      pointer stream holds the DVE/GPSIMD shared SBUF port).
  Device, per dst tile (bucket):
    - dma_gather pulls the unique x16 rows (fp16, 256 B each) HBM -> SBUF
      on one of 4 SWDGE queues (desc-gen on Q7 pairs is the kernel's
      bottleneck; 4 queues run desc-gen 4x parallel).
    - nc.sync HWDGE streams the bucket's rw tile (no Q7 involvement).
    - TensorE accumulates psum[c, m] += G_chunk.T @ rw_chunk over the
      bucket's chunks => axT tile = (A @ X)^T[:, tile] in PSUM; ACT copies
      psum -> axT in SBUF.
    - Phase 2 per tile: psum2 = axT_i.T @ W + ones.T @ b, DVE relu, DMA out.
"""

import numpy as np
from contextlib import ExitStack

import concourse.bass as bass
import concourse.bacc as bacc
import concourse.mybir as mybir
import concourse.tile as tile
from concourse import library_config
from concourse.bass_utils import run_bass_kernel_spmd

B, N, E, C = 8, 10000, 320000, 128

F16 = mybir.dt.float16
F32 = mybir.dt.float32
I16 = mybir.dt.int16


# ---------------------------------------------------------------- host prep

def balance_perm(rows, n_nodes, nt):
    """Renumber destination nodes so each 128-node dst tile receives ~equal
    edge counts: snake-assign nodes (sorted by in-degree desc) over nt
    tiles. Tiles get <=128 nodes; unused slots produce dead output rows the
    host ignores. Returns perm (old node id -> new slot id in [0, nt*128)).
    """
    deg = np.bincount(rows, minlength=n_nodes)
    order = np.argsort(-deg, kind="stable")
    n_pass = (n_nodes + nt - 1) // nt
    tile_seq = np.tile(np.concatenate([np.arange(nt), np.arange(nt)[::-1]]),
                       (n_pass + 1) // 2 + 1)[:n_nodes]
    slot_seq = np.repeat(np.arange(n_pass), nt)[:n_nodes]
    perm = np.empty(n_nodes, np.int64)
    perm[order] = tile_seq * 128 + slot_seq
    return perm


def dedup_prep(rows, cols, vals, nt, upb=None):
    """Per-dst-tile unique-source layout for the stream kernel.

    Each dst tile's unique sources occupy consecutive gather slots; R' rows
    are multi-hot (host-accumulated). Returns (uidx [nt, upb*128] int32
    gather indices, rw_rows list of (slot, m, val) arrays per tile) packed
    as flat arrays, plus upb if it was derived.
    """
    bucket = rows.astype(np.int64) >> 7
    trow = rows.astype(np.int64) & 127
    # unique (bucket, col) pairs; edges -> their unique slot
    key = bucket * 10**6 + cols.astype(np.int64)
    uniq, inv = np.unique(key, return_inverse=True)
    ubucket = uniq // 10**6
    ucol = uniq % 10**6
    ucounts = np.bincount(ubucket, minlength=nt)
    if upb is None:
        upb = (int(ucounts.max()) + 127) // 128
    starts = np.zeros(nt + 1, np.int64)
    np.cumsum(ucounts, out=starts[1:])
    # slot of each unique source within its bucket
    wslot = np.arange(len(uniq)) - starts[ubucket]
    uslot = ubucket * (upb * 128) + wslot          # global padded slot
    # -1 padding: the Q7 desc-gen trims trailing negative indices, so those
    # pad slots cost no descriptors. Restrict the -1s to each bucket's LAST
    # chunk (earlier pad slots gather node 0 against all-zero R' rows); the
    # kernel memsets that last chunk so trimmed slots read 0.0, never NaN.
    cols_p = np.full(nt * upb * 128, -1, np.int16)
    cols_p[uslot] = ucol.astype(np.int16)
    slot_in_bucket = np.arange(nt * upb * 128) % (upb * 128)
    zero_pad = (cols_p == -1) & (slot_in_bucket < (upb - 1) * 128)
    cols_p[zero_pad] = 0
    edge_slot = uslot[inv]                          # edge -> gather slot
    return cols_p, edge_slot, trow, upb


def prep_graph_v2(rows, cols, vals, nt, upb):
    """Dedup + stream prep: returns (colsw, rw, ucnt) for one graph."""
    cols_p, edge_slot, trow, _ = dedup_prep(rows, cols, vals, nt, upb)
    chunks = nt * upb
    colsw = np.tile(np.ascontiguousarray(cols_p.reshape(-1, 16).T), (8, 1))
    rw = np.zeros((128, chunks, 128), np.float32)
    p = edge_slot % 128
    k = edge_slot // 128
    np.add.at(rw, (p, k, trow), vals.astype(np.float32))
    ucnt = (cols_p.reshape(nt, upb * 128) >= 0).sum(axis=1).astype(np.int32)
    return colsw, np.ascontiguousarray(
        rw.reshape(128, chunks * 128).astype(np.float16)), ucnt


def max_unique_chunks(all_rows, all_cols, nt):
    mx = 0
    for rows, cols in zip(all_rows, all_cols):
        bucket = rows.astype(np.int64) >> 7
        key = bucket * 10**6 + cols.astype(np.int64)
        uniq = np.unique(key)
        ucounts = np.bincount(uniq // 10**6, minlength=nt)
        mx = max(mx, int(ucounts.max()))
    return (mx + 127) // 128


def prep_graph(rows, cols, vals, nt, cpb):
    """Bucket one graph's edges by destination tile, pad, build device layouts.

    Returns (colsw [128, EP/16] i16, trowsw [128, EP/128] f16,
             tvalsw [128, EP/128] f16) where EP = nt*cpb*128.
    """
    ep = nt * cpb * 128
    e = rows.shape[0]
    bucket = (rows.astype(np.int64) >> 7)
    order = np.argsort(bucket, kind="stable")
    sb = bucket[order]
    counts = np.bincount(bucket, minlength=nt)
    starts = np.zeros(nt + 1, np.int64)
    np.cumsum(counts, out=starts[1:])
    wbi = np.arange(e, dtype=np.int64) - starts[sb]
    pos = sb * (cpb * 128) + wbi

    cols_p = np.zeros(ep, np.int16)
    vals_p = np.zeros(ep, np.float32)
    trow_p = np.zeros(ep, np.float32)
    cols_p[pos] = cols[order].astype(np.int16)
    vals_p[pos] = vals[order].astype(np.float32)
    trow_p[pos] = (rows[order].astype(np.int64) - sb * 128).astype(np.float32)

    colsw = np.tile(np.ascontiguousarray(cols_p.reshape(-1, 16).T), (8, 1))
    trowsw = np.ascontiguousarray(trow_p.reshape(-1, 128).T)
    tvalsw = np.ascontiguousarray(vals_p.reshape(-1, 128).T)
    return colsw, trowsw, tvalsw


def max_bucket_chunks(all_rows, nt):
    """CPB = max over graphs/buckets of ceil(bucket_size/128)."""
    mx = 0
    for rows in all_rows:
        counts = np.bincount(rows.astype(np.int64) >> 7, minlength=nt)
        mx = max(mx, int(counts.max()))
    return (mx + 127) // 128


# ---------------------------------------------------------------- device code

def phase2_tile(nc, i, axT, wsb, bsb, ones, ps2, opool, out_d):
    """out[tile i] = relu(axT_i.T @ W + b)"""
    axT_i = axT[:, i * 128:(i + 1) * 128]
    ps2t = ps2.tile([128, 128], F32, tag="ps2")
    nc.tensor.matmul(ps2t[:], axT_i, wsb[:], start=True, stop=False)
    nc.tensor.matmul(ps2t[:], ones[:], bsb[:], start=False, stop=True)
    ot = opool.tile([128, 128], F32, tag="o")
    nc.vector.tensor_scalar(
        ot[:], ps2t[:], 0.0, None, op0=mybir.AluOpType.max,
    )
    nc.sync.dma_start(out_d[i * 128:(i + 1) * 128, :], ot[:])


def build_nc(n_nodes, nt, cpb, num_devices=8, reps=1, n_queues=4, mode="full",
             single_packet=False, interleave_p2=False, r_mode="dve2p",
             rw_engine="sync", rw_bufs=4):
    """Build the per-core bass program (same NEFF for all cores).

    reps > 1 repeats the whole compute (timing amortization only).
    n_queues: SWDGE queues; dma_gather desc-gen runs on Q7 core pair
    (2q, 2q+1), so round-robin queue_num parallelizes desc-gen 4x.
    mode: ablation switch — "full", "gather" (skip DVE+PE chunk work),
    "compute" (skip dma_gathers), "nodve" (skip tensor_scalar only).
    single_packet: coalesce each gather's CME descriptor stream into one
    packet (amortizes per-packet SDMA overhead for 256 B descriptors).
    interleave_p2: run each tile's phase-2 (axT@W+b, relu, store) right
    after its bucket accumulation instead of as a separate tail loop.
    r_mode: how the scaled one-hot R' tiles are built.
      "dve2p": DVE tensor_scalar, fp16 iota (4x_2P perf mode — locks GPSIMD
               out of the shared SBUF port during each op).
      "dve1x": DVE tensor_scalar, fp32 iota with odd 129-wide AP — forces
               1x perf mode, which never touches the shared port, so Q7
               desc-gen overlaps DVE.
      "split": alternate chunks between DVE (1x) and ACT (Square +
               Relu(-2t+vals) two-op build) to halve each sequencer's load.
      "stream": no on-chip build at all — host precomputes R' (fp16
               scaled one-hots, 256 B/edge) and the kernel streams it from
               HBM via HWDGE sync DMA, which never touches the Q7/DVE
               shared-port path. Trades ~2x HBM bytes for zero conflict.
    """
    chunks = nt * cpb
    ep = chunks * 128
    nc = bacc.Bacc(
        "TRN2",
        target_bir_lowering=False,
        debug=False,
        num_devices=num_devices,
        num_swdge_queues=n_queues,
    )

    x16_d = nc.dram_tensor("x16", [n_nodes, C], F16, kind="ExternalInput")
    colsw_d = nc.dram_tensor("colsw", [128, ep // 16], I16, kind="ExternalInput")
    if r_mode == "stream":
        rw_d = nc.dram_tensor("rw", [128, chunks * 128], F16, kind="ExternalInput")
        ucnt_d = nc.dram_tensor("ucnt", [1, nt], mybir.dt.int32,
                                kind="ExternalInput")
    else:
        trows_d = nc.dram_tensor("trows", [128, chunks], F32, kind="ExternalInput")
        tvals_d = nc.dram_tensor("tvals", [128, chunks], F32, kind="ExternalInput")
        iota_d = nc.dram_tensor("iota", [128, 130], F16, kind="ExternalInput")
    w_d = nc.dram_tensor("w", [C, C], F32, kind="ExternalInput")
    b_d = nc.dram_tensor("b", [1, C], F32, kind="ExternalInput")
    out_d = nc.dram_tensor("out", [nt * 128, C], F32, kind="ExternalOutput")

    with tile.TileContext(nc) as tc, ExitStack() as ctx:
        nc.gpsimd.load_library(library_config.mlp)
        const = ctx.enter_context(tc.tile_pool(name="const", bufs=1))
        gpool = ctx.enter_context(tc.tile_pool(name="g", bufs=2))
        rpool = ctx.enter_context(tc.tile_pool(name="r", bufs=12))
        rwpool = (ctx.enter_context(tc.tile_pool(name="rw", bufs=rw_bufs))
                  if r_mode == "stream" else None)
        ps1 = ctx.enter_context(tc.tile_pool(name="ps1", bufs=4, space="PSUM"))
        ps2 = ctx.enter_context(tc.tile_pool(name="ps2", bufs=2, space="PSUM"))
        opool = ctx.enter_context(tc.tile_pool(name="o", bufs=4))

        colsw = const.tile([128, ep // 16], I16, tag="colsw")
        nc.sync.dma_start(colsw[:], colsw_d[:, :])
        ucnt = ucnt_regs = None
        if r_mode == "stream":
            ucnt = const.tile([1, nt], mybir.dt.int32, tag="ucnt")
            nc.sync.dma_start(ucnt[:], ucnt_d[:, :])
            ucnt_regs = [nc.gpsimd.alloc_register(f"ucnt{q}")
                         for q in range(n_queues)]
        trows = tvals = iota = None
        if r_mode != "stream":
            trows = const.tile([128, chunks], F32, tag="trows")
            nc.sync.dma_start(trows[:], trows_d[:, :])
            tvals = const.tile([128, chunks], F32, tag="tvals")
            nc.sync.dma_start(tvals[:], tvals_d[:, :])
            iota = const.tile([128, 130], F16, tag="iota")
            nc.sync.dma_start(iota[:], iota_d[:, :])
        wsb = const.tile([C, C], F32, tag="w")
        nc.sync.dma_start(wsb[:], w_d[:, :])
        bsb = const.tile([1, C], F32, tag="b")
        nc.sync.dma_start(bsb[:], b_d[:, :])
        ones = const.tile([1, 128], F32, tag="ones")
        nc.vector.memset(ones[:], 1.0)
        iota32 = trowsn = None
        if r_mode in ("dve1x", "split"):
            # fp32 iota + odd-width APs force DVE 1x perf mode (no shared
            # SBUF port -> no GPSIMD desc-gen lockout).
            iota32 = const.tile([128, 130], F32, tag="iota32")
            nc.vector.tensor_scalar(
                iota32[:], iota[:, 0:130], 0.0, None, op0=mybir.AluOpType.add,
            )
        if r_mode == "split":
            trowsn = const.tile([128, chunks], F32, tag="trowsn")
            nc.vector.tensor_scalar(
                trowsn[:], trows[:], -1.0, None, op0=mybir.AluOpType.mult,
            )
        axT = const.tile([128, nt * 128], F32, tag="axT")
        if mode in ("gather", "gatherrw"):
            nc.vector.memset(axT[:], 0.0)
        gconst = rconst = None
        if mode == "compute":
            gconst = const.tile([128, n_queues * cpb, C], F16, tag="gconst")
            nc.vector.memset(gconst[:], 0.0)
        if mode == "nodve":
            rconst = const.tile([128, 132], F16, tag="rconst")
            nc.vector.memset(rconst[:], 0.0)

        NG = n_queues  # buckets per gather group (one per SWDGE queue)
        n_groups = (nt + NG - 1) // NG
        for _rep in range(reps):
          for grp in range(n_groups):
              # All of a group's gathers share one double-buffered group tile,
              # so their slot-WAR wait clears atomically and the 4 gathers
              # dispatch back-to-back -> desc-gen runs on all 4 Q7 core pairs.
              gb = gconst if mode == "compute" else gpool.tile(
                  [128, NG * cpb, C], F16, tag="g")
              if mode != "compute":
                  for q in range(NG):
                      i = grp * NG + q
                      if i >= nt:
                          continue
                      if ucnt_regs is not None:
                          nc.gpsimd.reg_load(ucnt_regs[q], ucnt[0:1, i:i + 1])
                          nreg = ucnt_regs[q]
                          # zero the -1-trimmed tail chunk before the gather
                          # overwrites the real slots (0.0, never NaN, into
                          # the matmul's contraction rows).
                          nc.vector.memset(
                              gb[:, (q + 1) * cpb - 1:(q + 1) * cpb, :], 0.0)
                      else:
                          nreg = cpb * 128
                      nc.gpsimd.dma_gather(
                          gb[:, q * cpb:(q + 1) * cpb, :],
                          x16_d[:, :],
                          colsw[:, i * cpb * 8:(i + 1) * cpb * 8],
                          num_idxs=cpb * 128,
                          num_idxs_reg=nreg,
                          elem_size=C,
                          single_packet=single_packet,
                          queue_num=q,
                      )
              if mode == "gather":
                  continue
              for q in range(NG):
                  i = grp * NG + q
                  if i >= nt:
                      continue
                  rwt = None
                  if r_mode == "stream" and mode != "nodve":
                      rwt = rwpool.tile([128, cpb * 128], F16, tag="rw")
                      rw_eng = nc.scalar if rw_engine == "scalar" else nc.sync
                      rw_eng.dma_start(
                          rwt[:], rw_d[:, i * cpb * 128:(i + 1) * cpb * 128])
                  if mode == "gatherrw":
                      continue
                  ps = ps1.tile([C, 128], F32, tag="ps1")
                  for k in range(cpb):
                      j = i * cpb + k
                      if mode == "nodve":
                          r = rconst
                      elif r_mode == "stream":
                          r = None
                      elif r_mode == "split" and k % 2 == 1:
                          # ACT two-op build: t = (iota - trow)^2, then
                          # r = relu(-2t + vals) = vals * onehot(trow).
                          r = rpool.tile([128, 132], F16, tag="r")
                          t = rpool.tile([128, 132], F16, tag="t")
                          nc.scalar.activation(
                              t[:, 0:128], iota[:, 0:128],
                              mybir.ActivationFunctionType.Square,
                              bias=trowsn[:, j:j + 1], scale=1.0,
                          )
                          nc.scalar.activation(
                              r[:, 0:128], t[:, 0:128],
                              mybir.ActivationFunctionType.Relu,
                              bias=tvals[:, j:j + 1], scale=-2.0,
                          )
                      elif r_mode in ("dve1x", "split"):
                          r = rpool.tile([128, 132], F16, tag="r")
                          nc.vector.tensor_scalar(
                              r[:, 0:129], iota32[:, 0:129],
                              trows[:, j:j + 1], tvals[:, j:j + 1],
                              op0=mybir.AluOpType.is_equal, op1=mybir.AluOpType.mult,
                          )
                      else:
                          r = rpool.tile([128, 132], F16, tag="r")
                          nc.vector.tensor_scalar(
                              r[:, 0:128], iota[:, 0:128],
                              trows[:, j:j + 1], tvals[:, j:j + 1],
                              op0=mybir.AluOpType.is_equal, op1=mybir.AluOpType.mult,
                          )
                      rhs = (rwt[:, k * 128:(k + 1) * 128]
                             if r_mode == "stream" and mode != "nodve"
                             else r[:, 0:128])
                      nc.tensor.matmul(
                          ps[:], gb[:, q * cpb + k, :], rhs,
                          start=(k == 0), stop=(k == cpb - 1),
                      )
                  axT_i = axT[:, i * 128:(i + 1) * 128]
                  nc.scalar.copy(axT_i, ps[:])
                  if interleave_p2:
                      phase2_tile(nc, i, axT, wsb, bsb, ones, ps2, opool, out_d)
          if not interleave_p2:
              for i in range(nt):
                  phase2_tile(nc, i, axT, wsb, bsb, ones, ps2, opool, out_d)

    nc.compile()
    return nc


# ---------------------------------------------------------------- entry point

_cache = {}

# Entry-point configuration: host-streamed R' (no on-chip one-hot build —
# DVE tensor_scalar locks GPSIMD out of the shared SBUF port and serializes
# against gather desc-gen) + per-dst-tile unique-source dedup (~20% fewer
# gather descriptors; desc-gen on Q7 is the gather bottleneck) + snake
# renumbering of destination nodes to even out per-tile edge counts.
KCFG = {"r_mode": "stream", "dedup": True, "balance": True, "rw_engine": "sync"}


def prep_all(x, rows, cols, vals, W, b):
    """Batch prep: balance dst nodes, size chunks, build in_maps.

    Returns (nt, cpb, in_maps, perms). perms is None when balancing is off;
    otherwise out_true[g][i] = out_dev[g][perms[g][i]].
    """
    nb, n_nodes, _ = x.shape
    nt = (n_nodes + 127) // 128
    perms = None
    if KCFG["balance"]:
        perms = [balance_perm(rows[g], n_nodes, nt) for g in range(nb)]
        rows = np.stack([perms[g][rows[g]] for g in range(nb)])
    if KCFG["dedup"]:
        cpb = max_unique_chunks([rows[g] for g in range(nb)],
                                [cols[g] for g in range(nb)], nt)
    else:
        cpb = max_bucket_chunks([rows[g] for g in range(nb)], nt)
    in_maps = make_in_maps(x, rows, cols, vals, W, b, nt, cpb,
                           r_mode=KCFG["r_mode"], dedup=KCFG["dedup"])
    return nt, cpb, in_maps, perms


def _get_nc(n_nodes, nt, cpb):
    key = (n_nodes, nt, cpb, KCFG["r_mode"], KCFG["rw_engine"])
    if key not in _cache:
        _cache[key] = build_nc(n_nodes, nt, cpb, r_mode=KCFG["r_mode"],
                               rw_engine=KCFG["rw_engine"])
    return _cache[key]


def make_in_maps(x, rows, cols, vals, W, b, nt, cpb, r_mode="dve2p",
                 dedup=False):
    nb = x.shape[0]
    iota_np = np.tile(np.arange(130, dtype=np.float16), (128, 1))
    in_maps = []
    for g in range(nb):
        m = {
            "x16": np.ascontiguousarray(x[g].astype(np.float16)),
            "w": np.ascontiguousarray(W.astype(np.float32)),
            "b": np.ascontiguousarray(b.astype(np.float32)[None, :]),
        }
        if dedup:
            assert r_mode == "stream"
            colsw, rww, ucnt = prep_graph_v2(rows[g], cols[g], vals[g], nt, cpb)
            m.update(colsw=colsw, rw=rww, ucnt=ucnt[None, :])
        else:
            colsw, trowsw, tvalsw = prep_graph(rows[g], cols[g], vals[g], nt, cpb)
            m["colsw"] = colsw
            if r_mode == "stream":
                m["rw"] = prep_rw(trowsw, tvalsw)
                m["ucnt"] = np.full((1, nt), cpb * 128, np.int32)
            else:
                m.update(trows=trowsw, tvals=tvalsw, iota=iota_np)
        in_maps.append(m)
    return in_maps


def prep_rw(trowsw, tvalsw):
    """Expand (trow, val) per edge slot into wrapped fp16 scaled one-hots.

    trowsw/tvalsw: [128, chunks] — edge slot (p, k). Output [128, chunks*128]
    where chunk k's columns [k*128, (k+1)*128) hold R'[e, m] = vals*(m==trow).
    """
    chunks = trowsw.shape[1]
    rw = np.zeros((128, chunks, 128), np.float16)
    p_idx = np.repeat(np.arange(128), chunks)
    k_idx = np.tile(np.arange(chunks), 128)
    rw[p_idx, k_idx, trowsw.astype(np.int64).ravel()] = tvalsw.ravel()
    return np.ascontiguousarray(rw.reshape(128, chunks * 128))


def kernel(x, rows, cols, vals, W, b, _trace=False):
    x = np.asarray(x)
    rows = np.asarray(rows)
    cols = np.asarray(cols)
    vals = np.asarray(vals)
    W = np.asarray(W)
    b = np.asarray(b)
    nb, n_nodes, _ = x.shape
    nt, cpb, in_maps, perms = prep_all(x, rows, cols, vals, W, b)
    nc = _get_nc(n_nodes, nt, cpb)
    res = run_bass_kernel_spmd(
        nc, in_maps, core_ids=list(range(nb)), trace=_trace,
    )
    if perms is None:
        out = np.stack([r["out"][:n_nodes] for r in res.results])
    else:
        out = np.stack([res.results[g]["out"][perms[g]] for g in range(nb)])
    out = out.astype(np.float32)
    if _trace:
        return out, res
    return out



# revision 62
# speedup vs baseline: 1.3022x; 1.3022x over previous
"""GCNN message-passing kernel for Trainium2 (8 NeuronCores, batch-parallel).

Reference computation per graph:
    ax  = segment_sum(vals[:, None] * x[cols], rows, N)   # sparse A @ x
    out = relu(ax @ W + b)

Sharding: one graph per NeuronCore (data parallel over batch, W/b replicated).

Per-core device strategy (KCFG defaults: stream + dedup + balance):
  Host prep (index marshaling only):
    - balance_perm: snake-renumber destination nodes by in-degree so every
      128-node dst tile gets ~equal edge counts (evens out per-queue
      desc-gen work; output rows un-permuted on the host afterwards).
    - Per dst tile, collect the UNIQUE source nodes (~3.4k of 4.1k edges);
      each unique source occupies one gather slot. colsw holds the slot ->
      source-row indices (int16, dma_gather wrapped layout), padded with -1
      in each tile's last chunk (the Q7 trims trailing negatives, so pad
      slots cost no descriptors).
    - rw: fp16 multi-hot scatter matrices, 128 cols per chunk; row s of
      chunk k accumulates vals of all edges from that slot's source into
      dst offset m. Streamed from HBM at run time (HWDGE; building these
      on-chip with DVE tensor_scalar stalls GPSIMD desc-gen: the scalar-
      pointer stream holds the DVE/GPSIMD shared SBUF port).
  Device, per dst tile (bucket):
    - dma_gather pulls the unique x16 rows (fp16, 256 B each) HBM -> SBUF
      on one of 4 SWDGE queues (desc-gen on Q7 pairs is the kernel's
      bottleneck; 4 queues run desc-gen 4x parallel).
    - nc.sync HWDGE streams the bucket's rw tile (no Q7 involvement).
    - TensorE accumulates psum[c, m] += G_chunk.T @ rw_chunk over the
      bucket's chunks => axT tile = (A @ X)^T[:, tile] in PSUM; ACT copies
      psum -> axT in SBUF.
    - Phase 2 per tile: psum2 = axT_i.T @ W + ones.T @ b, DVE relu, DMA out.
"""

import numpy as np
from contextlib import ExitStack

import concourse.bass as bass
import concourse.bacc as bacc
import concourse.mybir as mybir
import concourse.tile as tile
from concourse import library_config
from concourse.bass_utils import run_bass_kernel_spmd

B, N, E, C = 8, 10000, 320000, 128

F16 = mybir.dt.float16
F32 = mybir.dt.float32
I16 = mybir.dt.int16


# ---------------------------------------------------------------- host prep

def balance_perm(rows, n_nodes, nt):
    """Renumber destination nodes so each 128-node dst tile receives ~equal
    edge counts: snake-assign nodes (sorted by in-degree desc) over nt
    tiles. Tiles get <=128 nodes; unused slots produce dead output rows the
    host ignores. Returns perm (old node id -> new slot id in [0, nt*128)).
    """
    deg = np.bincount(rows, minlength=n_nodes)
    order = np.argsort(-deg, kind="stable")
    n_pass = (n_nodes + nt - 1) // nt
    tile_seq = np.tile(np.concatenate([np.arange(nt), np.arange(nt)[::-1]]),
                       (n_pass + 1) // 2 + 1)[:n_nodes]
    slot_seq = np.repeat(np.arange(n_pass), nt)[:n_nodes]
    perm = np.empty(n_nodes, np.int64)
    perm[order] = tile_seq * 128 + slot_seq
    return perm


def dedup_prep(rows, cols, vals, nt, upb=None):
    """Per-dst-tile unique-source layout for the stream kernel.

    Each dst tile's unique sources occupy consecutive gather slots; R' rows
    are multi-hot (host-accumulated). Returns (uidx [nt, upb*128] int32
    gather indices, rw_rows list of (slot, m, val) arrays per tile) packed
    as flat arrays, plus upb if it was derived.
    """
    bucket = rows.astype(np.int64) >> 7
    trow = rows.astype(np.int64) & 127
    # unique (bucket, col) pairs; edges -> their unique slot
    key = bucket * 10**6 + cols.astype(np.int64)
    uniq, inv = np.unique(key, return_inverse=True)
    ubucket = uniq // 10**6
    ucol = uniq % 10**6
    ucounts = np.bincount(ubucket, minlength=nt)
    if upb is None:
        upb = (int(ucounts.max()) + 127) // 128
    starts = np.zeros(nt + 1, np.int64)
    np.cumsum(ucounts, out=starts[1:])
    # slot of each unique source within its bucket
    wslot = np.arange(len(uniq)) - starts[ubucket]
    uslot = ubucket * (upb * 128) + wslot          # global padded slot
    # -1 padding: the Q7 desc-gen trims trailing negative indices, so those
    # pad slots cost no descriptors. Restrict the -1s to each bucket's LAST
    # chunk (earlier pad slots gather node 0 against all-zero R' rows); the
    # kernel memsets that last chunk so trimmed slots read 0.0, never NaN.
    cols_p = np.full(nt * upb * 128, -1, np.int16)
    cols_p[uslot] = ucol.astype(np.int16)
    slot_in_bucket = np.arange(nt * upb * 128) % (upb * 128)
    zero_pad = (cols_p == -1) & (slot_in_bucket < (upb - 1) * 128)
    cols_p[zero_pad] = 0
    edge_slot = uslot[inv]                          # edge -> gather slot
    return cols_p, edge_slot, trow, upb


def prep_graph_v2(rows, cols, vals, nt, upb):
    """Dedup + stream prep: returns (colsw, rw, ucnt) for one graph."""
    cols_p, edge_slot, trow, _ = dedup_prep(rows, cols, vals, nt, upb)
    chunks = nt * upb
    colsw = np.tile(np.ascontiguousarray(cols_p.reshape(-1, 16).T), (8, 1))
    rw = np.zeros((128, chunks, 128), np.float32)
    p = edge_slot % 128
    k = edge_slot // 128
    np.add.at(rw, (p, k, trow), vals.astype(np.float32))
    ucnt = (cols_p.reshape(nt, upb * 128) >= 0).sum(axis=1).astype(np.int32)
    return colsw, np.ascontiguousarray(
        rw.reshape(128, chunks * 128).astype(np.float16)), ucnt


def _pair_split(rows, cols, n_nodes, nt):
    """Per-bucket unique sources split into adjacent pairs (2k, 2k+1) both
    needed vs singles. Returns per-bucket (pk arrays, sg arrays)."""
    bucket = rows.astype(np.int64) >> 7
    key = bucket * 10**6 + cols.astype(np.int64)
    uniq = np.unique(key)
    ub = uniq // 10**6
    uc = uniq % 10**6
    starts = np.searchsorted(ub, np.arange(nt + 1))
    out = []
    present = np.zeros(n_nodes + 1, bool)
    for b in range(nt):
        us = uc[starts[b]:starts[b + 1]]
        present[us] = True
        both = present[us & ~1] & present[us | 1]
        pk = (us[both & ((us & 1) == 0)] >> 1).astype(np.int64)
        sg = us[~both]
        present[us] = False
        out.append((pk, sg))
    return out


def max_pair_chunks(all_rows, all_cols, n_nodes, nt):
    """(pa, pb) = max chunk counts for pair / single gathers."""
    mp = ms = 0
    for rows, cols in zip(all_rows, all_cols):
        for pk, sg in _pair_split(rows, cols, n_nodes, nt):
            mp = max(mp, len(pk))
            ms = max(ms, len(sg))
    return (mp + 127) // 128, (ms + 127) // 128


def prep_graph_v3(rows, cols, vals, n_nodes, nt, pa, pb):
    """Pairs-mixed stream prep: (colswp, colsws, rw, pcnt, scnt).

    Gather 1 fetches 512 B pairs x16[2k:2k+2] for pairs both needed by the
    bucket; gather 2 fetches 256 B singles. Contraction chunk order per
    bucket: [pair chunk 0 half 0, half 1, pair chunk 1 half 0, ...] then
    single chunks; rw rows are multi-hot per slot's source.
    """
    cpb = 2 * pa + pb
    trow = rows.astype(np.int64) & 127
    bucket = rows.astype(np.int64) >> 7
    idxp = np.full((nt, pa * 128), -1, np.int16)
    idxs_ = np.full((nt, pb * 128), -1, np.int16)
    chunk_of = np.zeros((nt, n_nodes), np.int16)
    part_of = np.zeros((nt, n_nodes), np.int16)
    pcnt = np.zeros(nt, np.int32)
    scnt = np.zeros(nt, np.int32)
    for b, (pk, sg) in enumerate(_pair_split(rows, cols, n_nodes, nt)):
        idxp[b, :len(pk)] = pk.astype(np.int16)
        idxs_[b, :len(sg)] = sg.astype(np.int16)
        t = np.arange(len(pk))
        chunk_of[b, 2 * pk] = (2 * (t // 128)).astype(np.int16)
        chunk_of[b, 2 * pk + 1] = (2 * (t // 128) + 1).astype(np.int16)
        part_of[b, 2 * pk] = (t % 128).astype(np.int16)
        part_of[b, 2 * pk + 1] = (t % 128).astype(np.int16)
        ts = np.arange(len(sg))
        chunk_of[b, sg] = (2 * pa + ts // 128).astype(np.int16)
        part_of[b, sg] = (ts % 128).astype(np.int16)
    # 0-pad all but each bucket's last chunk so -1s stay trailing
    for arr, nch in ((idxp, pa), (idxs_, pb)):
        sl = np.arange(nch * 128)
        zp = (arr == -1) & (sl[None, :] < (nch - 1) * 128)
        arr[zp] = 0
    pcnt = (idxp >= 0).sum(axis=1).astype(np.int32)
    scnt = (idxs_ >= 0).sum(axis=1).astype(np.int32)
    rw = np.zeros((128, nt * cpb, 128), np.float32)
    j = chunk_of[bucket, cols] + bucket * cpb
    p = part_of[bucket, cols]
    np.add.at(rw, (p, j, trow), vals.astype(np.float32))
    colswp = np.tile(np.ascontiguousarray(idxp.reshape(-1, 16).T), (8, 1))
    colsws = np.tile(np.ascontiguousarray(idxs_.reshape(-1, 16).T), (8, 1))
    return (colswp, colsws,
            np.ascontiguousarray(rw.reshape(128, nt * cpb * 128).astype(np.float16)),
            pcnt, scnt)


def max_unique_chunks(all_rows, all_cols, nt):
    mx = 0
    for rows, cols in zip(all_rows, all_cols):
        bucket = rows.astype(np.int64) >> 7
        key = bucket * 10**6 + cols.astype(np.int64)
        uniq = np.unique(key)
        ucounts = np.bincount(uniq // 10**6, minlength=nt)
        mx = max(mx, int(ucounts.max()))
    return (mx + 127) // 128


def prep_graph(rows, cols, vals, nt, cpb):
    """Bucket one graph's edges by destination tile, pad, build device layouts.

    Returns (colsw [128, EP/16] i16, trowsw [128, EP/128] f16,
             tvalsw [128, EP/128] f16) where EP = nt*cpb*128.
    """
    ep = nt * cpb * 128
    e = rows.shape[0]
    bucket = (rows.astype(np.int64) >> 7)
    order = np.argsort(bucket, kind="stable")
    sb = bucket[order]
    counts = np.bincount(bucket, minlength=nt)
    starts = np.zeros(nt + 1, np.int64)
    np.cumsum(counts, out=starts[1:])
    wbi = np.arange(e, dtype=np.int64) - starts[sb]
    pos = sb * (cpb * 128) + wbi

    cols_p = np.zeros(ep, np.int16)
    vals_p = np.zeros(ep, np.float32)
    trow_p = np.zeros(ep, np.float32)
    cols_p[pos] = cols[order].astype(np.int16)
    vals_p[pos] = vals[order].astype(np.float32)
    trow_p[pos] = (rows[order].astype(np.int64) - sb * 128).astype(np.float32)

    colsw = np.tile(np.ascontiguousarray(cols_p.reshape(-1, 16).T), (8, 1))
    trowsw = np.ascontiguousarray(trow_p.reshape(-1, 128).T)
    tvalsw = np.ascontiguousarray(vals_p.reshape(-1, 128).T)
    return colsw, trowsw, tvalsw


def max_bucket_chunks(all_rows, nt):
    """CPB = max over graphs/buckets of ceil(bucket_size/128)."""
    mx = 0
    for rows in all_rows:
        counts = np.bincount(rows.astype(np.int64) >> 7, minlength=nt)
        mx = max(mx, int(counts.max()))
    return (mx + 127) // 128


# ---------------------------------------------------------------- device code

def phase2_tile(nc, i, axT, wsb, bsb, ones, ps2, opool, out_d):
    """out[tile i] = relu(axT_i.T @ W + b)"""
    axT_i = axT[:, i * 128:(i + 1) * 128]
    ps2t = ps2.tile([128, 128], F32, tag="ps2")
    nc.tensor.matmul(ps2t[:], axT_i, wsb[:], start=True, stop=False)
    nc.tensor.matmul(ps2t[:], ones[:], bsb[:], start=False, stop=True)
    ot = opool.tile([128, 128], F32, tag="o")
    nc.vector.tensor_scalar(
        ot[:], ps2t[:], 0.0, None, op0=mybir.AluOpType.max,
    )
    nc.sync.dma_start(out_d[i * 128:(i + 1) * 128, :], ot[:])


def build_nc(n_nodes, nt, cpb, num_devices=8, reps=1, n_queues=4, mode="full",
             single_packet=False, interleave_p2=False, r_mode="dve2p",
             rw_engine="sync", rw_bufs=4, ng=None, pairs=False, pa=0, pb=0):
    """Build the per-core bass program (same NEFF for all cores).

    reps > 1 repeats the whole compute (timing amortization only).
    n_queues: SWDGE queues; dma_gather desc-gen runs on Q7 core pair
    (2q, 2q+1), so round-robin queue_num parallelizes desc-gen 4x.
    mode: ablation switch — "full", "gather" (skip DVE+PE chunk work),
    "compute" (skip dma_gathers), "nodve" (skip tensor_scalar only).
    single_packet: coalesce each gather's CME descriptor stream into one
    packet (amortizes per-packet SDMA overhead for 256 B descriptors).
    interleave_p2: run each tile's phase-2 (axT@W+b, relu, store) right
    after its bucket accumulation instead of as a separate tail loop.
    r_mode: how the scaled one-hot R' tiles are built.
      "dve2p": DVE tensor_scalar, fp16 iota (4x_2P perf mode — locks GPSIMD
               out of the shared SBUF port during each op).
      "dve1x": DVE tensor_scalar, fp32 iota with odd 129-wide AP — forces
               1x perf mode, which never touches the shared port, so Q7
               desc-gen overlaps DVE.
      "split": alternate chunks between DVE (1x) and ACT (Square +
               Relu(-2t+vals) two-op build) to halve each sequencer's load.
      "stream": no on-chip build at all — host precomputes R' (fp16
               scaled one-hots, 256 B/edge) and the kernel streams it from
               HBM via HWDGE sync DMA, which never touches the Q7/DVE
               shared-port path. Trades ~2x HBM bytes for zero conflict.
    """
    chunks = nt * cpb
    ep = chunks * 128
    nc = bacc.Bacc(
        "TRN2",
        target_bir_lowering=False,
        debug=False,
        num_devices=num_devices,
        num_swdge_queues=n_queues,
    )

    if pairs:
        assert r_mode == "stream" and cpb == 2 * pa + pb and n_nodes % 2 == 0
    x16_d = nc.dram_tensor("x16", [n_nodes, C], F16, kind="ExternalInput")
    if pairs:
        x16p_d = nc.dram_tensor("x16p", [n_nodes // 2, 2 * C], F16,
                                kind="ExternalInput")
        colswp_d = nc.dram_tensor("colswp", [128, nt * pa * 8], I16,
                                  kind="ExternalInput")
        colsws_d = nc.dram_tensor("colsws", [128, nt * pb * 8], I16,
                                  kind="ExternalInput")
        pcnt_d = nc.dram_tensor("pcnt", [1, nt], mybir.dt.int32,
                                kind="ExternalInput")
        scnt_d = nc.dram_tensor("scnt", [1, nt], mybir.dt.int32,
                                kind="ExternalInput")
    else:
        colsw_d = nc.dram_tensor("colsw", [128, ep // 16], I16,
                                 kind="ExternalInput")
    if r_mode == "stream":
        rw_d = nc.dram_tensor("rw", [128, chunks * 128], F16, kind="ExternalInput")
        if not pairs:
            ucnt_d = nc.dram_tensor("ucnt", [1, nt], mybir.dt.int32,
                                    kind="ExternalInput")
    else:
        trows_d = nc.dram_tensor("trows", [128, chunks], F32, kind="ExternalInput")
        tvals_d = nc.dram_tensor("tvals", [128, chunks], F32, kind="ExternalInput")
        iota_d = nc.dram_tensor("iota", [128, 130], F16, kind="ExternalInput")
    w_d = nc.dram_tensor("w", [C, C], F32, kind="ExternalInput")
    b_d = nc.dram_tensor("b", [1, C], F32, kind="ExternalInput")
    out_d = nc.dram_tensor("out", [nt * 128, C], F32, kind="ExternalOutput")

    with tile.TileContext(nc) as tc, ExitStack() as ctx:
        nc.gpsimd.load_library(library_config.mlp)
        const = ctx.enter_context(tc.tile_pool(name="const", bufs=1))
        gpool = ctx.enter_context(tc.tile_pool(name="g", bufs=2))
        rpool = ctx.enter_context(tc.tile_pool(name="r", bufs=12))
        rwpool = (ctx.enter_context(tc.tile_pool(name="rw", bufs=rw_bufs))
                  if r_mode == "stream" else None)
        ps1 = ctx.enter_context(tc.tile_pool(name="ps1", bufs=4, space="PSUM"))
        ps2 = ctx.enter_context(tc.tile_pool(name="ps2", bufs=2, space="PSUM"))
        opool = ctx.enter_context(tc.tile_pool(name="o", bufs=4))

        colsw = ucnt = ucnt_regs = None
        colswp = colsws = pcnt_sb = scnt_sb = scnt_regs = None
        if pairs:
            colswp = const.tile([128, nt * pa * 8], I16, tag="colswp")
            nc.sync.dma_start(colswp[:], colswp_d[:, :])
            colsws = const.tile([128, nt * pb * 8], I16, tag="colsws")
            nc.sync.dma_start(colsws[:], colsws_d[:, :])
            pcnt_sb = const.tile([1, nt], mybir.dt.int32, tag="pcnt")
            nc.sync.dma_start(pcnt_sb[:], pcnt_d[:, :])
            scnt_sb = const.tile([1, nt], mybir.dt.int32, tag="scnt")
            nc.sync.dma_start(scnt_sb[:], scnt_d[:, :])
            ucnt_regs = [nc.gpsimd.alloc_register(f"pcr{q}")
                         for q in range(n_queues)]
            scnt_regs = [nc.gpsimd.alloc_register(f"scr{q}")
                         for q in range(n_queues)]
        else:
            colsw = const.tile([128, ep // 16], I16, tag="colsw")
            nc.sync.dma_start(colsw[:], colsw_d[:, :])
            if r_mode == "stream":
                ucnt = const.tile([1, nt], mybir.dt.int32, tag="ucnt")
                nc.sync.dma_start(ucnt[:], ucnt_d[:, :])
                ucnt_regs = [nc.gpsimd.alloc_register(f"ucnt{q}")
                             for q in range(n_queues)]
        trows = tvals = iota = None
        if r_mode != "stream":
            trows = const.tile([128, chunks], F32, tag="trows")
            nc.sync.dma_start(trows[:], trows_d[:, :])
            tvals = const.tile([128, chunks], F32, tag="tvals")
            nc.sync.dma_start(tvals[:], tvals_d[:, :])
            iota = const.tile([128, 130], F16, tag="iota")
            nc.sync.dma_start(iota[:], iota_d[:, :])
        wsb = const.tile([C, C], F32, tag="w")
        nc.sync.dma_start(wsb[:], w_d[:, :])
        bsb = const.tile([1, C], F32, tag="b")
        nc.sync.dma_start(bsb[:], b_d[:, :])
        ones = const.tile([1, 128], F32, tag="ones")
        nc.vector.memset(ones[:], 1.0)
        iota32 = trowsn = None
        if r_mode in ("dve1x", "split"):
            # fp32 iota + odd-width APs force DVE 1x perf mode (no shared
            # SBUF port -> no GPSIMD desc-gen lockout).
            iota32 = const.tile([128, 130], F32, tag="iota32")
            nc.vector.tensor_scalar(
                iota32[:], iota[:, 0:130], 0.0, None, op0=mybir.AluOpType.add,
            )
        if r_mode == "split":
            trowsn = const.tile([128, chunks], F32, tag="trowsn")
            nc.vector.tensor_scalar(
                trowsn[:], trows[:], -1.0, None, op0=mybir.AluOpType.mult,
            )
        axT = const.tile([128, nt * 128], F32, tag="axT")
        if mode in ("gather", "gatherrw"):
            nc.vector.memset(axT[:], 0.0)
        gconst = rconst = None
        if mode == "compute":
            gconst = const.tile([128, (ng or n_queues) * cpb, C], F16,
                                tag="gconst")
            nc.vector.memset(gconst[:], 0.0)
        if mode == "nodve":
            rconst = const.tile([128, 132], F16, tag="rconst")
            nc.vector.memset(rconst[:], 0.0)

        # buckets per gather group; queues assigned i % n_queues so any group
        # size keeps all SWDGE queues busy across in-flight groups.
        NG = ng if ng is not None else n_queues
        n_groups = (nt + NG - 1) // NG
        for _rep in range(reps):
          for grp in range(n_groups):
              # All of a group's gathers share one double-buffered group tile,
              # so their slot-WAR wait clears atomically and the 4 gathers
              # dispatch back-to-back -> desc-gen runs on all 4 Q7 core pairs.
              gb = gp = gs = None
              if pairs:
                  gp = gpool.tile([128, NG * pa, 2 * C], F16, tag="gp")
                  gs = gpool.tile([128, NG * pb, C], F16, tag="gs")
              elif mode == "compute":
                  gb = gconst
              else:
                  gb = gpool.tile([128, NG * cpb, C], F16, tag="g")
              if mode != "compute":
                  for q in range(NG):
                      i = grp * NG + q
                      if i >= nt:
                          continue
                      qq = i % n_queues
                      if pairs:
                          nc.gpsimd.reg_load(ucnt_regs[qq], pcnt_sb[0:1, i:i + 1])
                          nc.vector.memset(
                              gp[:, (q + 1) * pa - 1:(q + 1) * pa, :], 0.0)
                          nc.gpsimd.dma_gather(
                              gp[:, q * pa:(q + 1) * pa, :],
                              x16p_d[:, :],
                              colswp[:, i * pa * 8:(i + 1) * pa * 8],
                              num_idxs=pa * 128,
                              num_idxs_reg=ucnt_regs[qq],
                              elem_size=2 * C,
                              single_packet=single_packet,
                              queue_num=qq,
                          )
                          nc.gpsimd.reg_load(scnt_regs[qq], scnt_sb[0:1, i:i + 1])
                          nc.vector.memset(
                              gs[:, (q + 1) * pb - 1:(q + 1) * pb, :], 0.0)
                          nc.gpsimd.dma_gather(
                              gs[:, q * pb:(q + 1) * pb, :],
                              x16_d[:, :],
                              colsws[:, i * pb * 8:(i + 1) * pb * 8],
                              num_idxs=pb * 128,
                              num_idxs_reg=scnt_regs[qq],
                              elem_size=C,
                              single_packet=single_packet,
                              queue_num=qq,
                          )
                          continue
                      if ucnt_regs is not None:
                          nc.gpsimd.reg_load(ucnt_regs[qq], ucnt[0:1, i:i + 1])
                          nreg = ucnt_regs[qq]
                          # zero the -1-trimmed tail chunk before the gather
                          # overwrites the real slots (0.0, never NaN, into
                          # the matmul's contraction rows).
                          nc.vector.memset(
                              gb[:, (q + 1) * cpb - 1:(q + 1) * cpb, :], 0.0)
                      else:
                          nreg = cpb * 128
                      nc.gpsimd.dma_gather(
                          gb[:, q * cpb:(q + 1) * cpb, :],
                          x16_d[:, :],
                          colsw[:, i * cpb * 8:(i + 1) * cpb * 8],
                          num_idxs=cpb * 128,
                          num_idxs_reg=nreg,
                          elem_size=C,
                          single_packet=single_packet,
                          queue_num=qq,
                      )
              if mode == "gather":
                  continue
              for q in range(NG):
                  i = grp * NG + q
                  if i >= nt:
                      continue
                  rwt = None
                  if r_mode == "stream" and mode != "nodve":
                      rwt = rwpool.tile([128, cpb * 128], F16, tag="rw")
                      if rw_engine == "alt":
                          # alternate the two physical HWDGE rings
                          # (qSPDynamicHW / qActDynamicHW) so the FIFO-
                          # serialized rw wire time splits across both.
                          rw_eng = nc.scalar if i % 2 else nc.sync
                      elif rw_engine == "scalar":
                          rw_eng = nc.scalar
                      else:
                          rw_eng = nc.sync
                      rw_eng.dma_start(
                          rwt[:], rw_d[:, i * cpb * 128:(i + 1) * cpb * 128])
                  if mode == "gatherrw":
                      continue
                  ps = ps1.tile([C, 128], F32, tag="ps1")
                  for k in range(cpb):
                      j = i * cpb + k
                      if mode == "nodve":
                          r = rconst
                      elif r_mode == "stream":
                          r = None
                      elif r_mode == "split" and k % 2 == 1:
                          # ACT two-op build: t = (iota - trow)^2, then
                          # r = relu(-2t + vals) = vals * onehot(trow).
                          r = rpool.tile([128, 132], F16, tag="r")
                          t = rpool.tile([128, 132], F16, tag="t")
                          nc.scalar.activation(
                              t[:, 0:128], iota[:, 0:128],
                              mybir.ActivationFunctionType.Square,
                              bias=trowsn[:, j:j + 1], scale=1.0,
                          )
                          nc.scalar.activation(
                              r[:, 0:128], t[:, 0:128],
                              mybir.ActivationFunctionType.Relu,
                              bias=tvals[:, j:j + 1], scale=-2.0,
                          )
                      elif r_mode in ("dve1x", "split"):
                          r = rpool.tile([128, 132], F16, tag="r")
                          nc.vector.tensor_scalar(
                              r[:, 0:129], iota32[:, 0:129],
                              trows[:, j:j + 1], tvals[:, j:j + 1],
                              op0=mybir.AluOpType.is_equal, op1=mybir.AluOpType.mult,
                          )
                      else:
                          r = rpool.tile([128, 132], F16, tag="r")
                          nc.vector.tensor_scalar(
                              r[:, 0:128], iota[:, 0:128],
                              trows[:, j:j + 1], tvals[:, j:j + 1],
                              op0=mybir.AluOpType.is_equal, op1=mybir.AluOpType.mult,
                          )
                      rhs = (rwt[:, k * 128:(k + 1) * 128]
                             if r_mode == "stream" and mode != "nodve"
                             else r[:, 0:128])
                      if pairs:
                          if k < 2 * pa:
                              pc, h = k // 2, k % 2
                              lhsT = gp[:, q * pa + pc, h * C:(h + 1) * C]
                          else:
                              lhsT = gs[:, q * pb + (k - 2 * pa), :]
                      else:
                          lhsT = gb[:, q * cpb + k, :]
                      nc.tensor.matmul(
                          ps[:], lhsT, rhs,
                          start=(k == 0), stop=(k == cpb - 1),
                      )
                  axT_i = axT[:, i * 128:(i + 1) * 128]
                  nc.scalar.copy(axT_i, ps[:])
                  if interleave_p2:
                      phase2_tile(nc, i, axT, wsb, bsb, ones, ps2, opool, out_d)
          if not interleave_p2:
              for i in range(nt):
                  phase2_tile(nc, i, axT, wsb, bsb, ones, ps2, opool, out_d)

    nc.compile()
    return nc


# ---------------------------------------------------------------- entry point

_cache = {}

# Entry-point configuration: host-streamed R' (no on-chip one-hot build —
# DVE tensor_scalar locks GPSIMD out of the shared SBUF port and serializes
# against gather desc-gen) + per-dst-tile unique-source dedup (~20% fewer
# gather descriptors; desc-gen on Q7 is the gather bottleneck) + snake
# renumbering of destination nodes to even out per-tile edge counts.
KCFG = {"r_mode": "stream", "dedup": True, "balance": True, "rw_engine": "sync",
        "pairs": False}


def build_cfg(n_nodes, nt, size, num_devices=8, reps=1):
    """build_nc with KCFG applied; size is (pa, pb) when KCFG['pairs']."""
    kw = dict(r_mode=KCFG["r_mode"], rw_engine=KCFG["rw_engine"],
              num_devices=num_devices, reps=reps)
    if KCFG.get("pairs"):
        pa, pb = size
        return build_nc(n_nodes, nt, 2 * pa + pb, pairs=True, pa=pa, pb=pb,
                        **kw)
    return build_nc(n_nodes, nt, size, **kw)


def prep_all(x, rows, cols, vals, W, b):
    """Batch prep: balance dst nodes, size chunks, build in_maps.

    Returns (nt, cpb, in_maps, perms). perms is None when balancing is off;
    otherwise out_true[g][i] = out_dev[g][perms[g][i]].
    """
    nb, n_nodes, _ = x.shape
    nt = (n_nodes + 127) // 128
    perms = None
    if KCFG["balance"]:
        perms = [balance_perm(rows[g], n_nodes, nt) for g in range(nb)]
        rows = np.stack([perms[g][rows[g]] for g in range(nb)])
    if KCFG.get("pairs"):
        pa, pb = max_pair_chunks([rows[g] for g in range(nb)],
                                 [cols[g] for g in range(nb)], n_nodes, nt)
        in_maps = []
        for g in range(nb):
            colswp, colsws, rww, pcnt, scnt = prep_graph_v3(
                rows[g], cols[g], vals[g], n_nodes, nt, pa, pb)
            x16 = np.ascontiguousarray(x[g].astype(np.float16))
            in_maps.append({
                "x16": x16,
                "x16p": np.ascontiguousarray(x16.reshape(n_nodes // 2, 256)),
                "colswp": colswp, "colsws": colsws, "rw": rww,
                "pcnt": pcnt[None, :], "scnt": scnt[None, :],
                "w": np.ascontiguousarray(W.astype(np.float32)),
                "b": np.ascontiguousarray(b.astype(np.float32)[None, :]),
            })
        return nt, (pa, pb), in_maps, perms
    if KCFG["dedup"]:
        cpb = max_unique_chunks([rows[g] for g in range(nb)],
                                [cols[g] for g in range(nb)], nt)
    else:
        cpb = max_bucket_chunks([rows[g] for g in range(nb)], nt)
    in_maps = make_in_maps(x, rows, cols, vals, W, b, nt, cpb,
                           r_mode=KCFG["r_mode"], dedup=KCFG["dedup"])
    return nt, cpb, in_maps, perms


def _get_nc(n_nodes, nt, size):
    key = (n_nodes, nt, size, KCFG["r_mode"], KCFG["rw_engine"],
           KCFG.get("pairs", False))
    if key not in _cache:
        _cache[key] = build_cfg(n_nodes, nt, size)
    return _cache[key]


def make_in_maps(x, rows, cols, vals, W, b, nt, cpb, r_mode="dve2p",
                 dedup=False):
    nb = x.shape[0]
    iota_np = np.tile(np.arange(130, dtype=np.float16), (128, 1))
    in_maps = []
    for g in range(nb):
        m = {
            "x16": np.ascontiguousarray(x[g].astype(np.float16)),
            "w": np.ascontiguousarray(W.astype(np.float32)),
            "b": np.ascontiguousarray(b.astype(np.float32)[None, :]),
        }
        if dedup:
            assert r_mode == "stream"
            colsw, rww, ucnt = prep_graph_v2(rows[g], cols[g], vals[g], nt, cpb)
            m.update(colsw=colsw, rw=rww, ucnt=ucnt[None, :])
        else:
            colsw, trowsw, tvalsw = prep_graph(rows[g], cols[g], vals[g], nt, cpb)
            m["colsw"] = colsw
            if r_mode == "stream":
                m["rw"] = prep_rw(trowsw, tvalsw)
                m["ucnt"] = np.full((1, nt), cpb * 128, np.int32)
            else:
                m.update(trows=trowsw, tvals=tvalsw, iota=iota_np)
        in_maps.append(m)
    return in_maps


def prep_rw(trowsw, tvalsw):
    """Expand (trow, val) per edge slot into wrapped fp16 scaled one-hots.

    trowsw/tvalsw: [128, chunks] — edge slot (p, k). Output [128, chunks*128]
    where chunk k's columns [k*128, (k+1)*128) hold R'[e, m] = vals*(m==trow).
    """
    chunks = trowsw.shape[1]
    rw = np.zeros((128, chunks, 128), np.float16)
    p_idx = np.repeat(np.arange(128), chunks)
    k_idx = np.tile(np.arange(chunks), 128)
    rw[p_idx, k_idx, trowsw.astype(np.int64).ravel()] = tvalsw.ravel()
    return np.ascontiguousarray(rw.reshape(128, chunks * 128))


def kernel(x, rows, cols, vals, W, b, _trace=False):
    x = np.asarray(x)
    rows = np.asarray(rows)
    cols = np.asarray(cols)
    vals = np.asarray(vals)
    W = np.asarray(W)
    b = np.asarray(b)
    nb, n_nodes, _ = x.shape
    nt, cpb, in_maps, perms = prep_all(x, rows, cols, vals, W, b)
    nc = _get_nc(n_nodes, nt, cpb)
    res = run_bass_kernel_spmd(
        nc, in_maps, core_ids=list(range(nb)), trace=_trace,
    )
    if perms is None:
        out = np.stack([r["out"][:n_nodes] for r in res.results])
    else:
        out = np.stack([res.results[g]["out"][perms[g]] for g in range(nb)])
    out = out.astype(np.float32)
    if _trace:
        return out, res
    return out

